# revision 34
# baseline (speedup 1.0000x reference)
"""Trainium2 Bass kernel for gated 1x1-conv attention (dense_transformer).

Problem structure (B=4, C=3, H=W=64, heads=3 => c_h=1): attention logits are
rank-1: att[n] = softmax_m(q_n * k_m) @ v over N=4096 pixels; a luma gate
scales q; the 1x1 convs are 3x3 channel mixes.

Sharding: 8 cores = (batch b = j//2) x (query-pixel half = j%2); each core
produces the full RGB output for its 2048 query pixels. No collectives.

Algorithm (Gaussian-quadrature factorization of the exp kernel): over a
T=128 grid t_j with spacing hg and sigma ~= hg,
  e^{q k} ~= C(k) sum_j e^{-(q-t_j)^2/(2 s^2)} e^{t_j k}
(aliasing error ~e^{-2 pi^2 (s/hg)^2}; C(k)=e^{s^2k^2/2} cancels in the
softmax ratio up to a negligible reweighting). This collapses the N x N
attention to N x T + T x N:
  grid:  gnum[j] = sum_m v_m e^{t_j k_m},  gden[j] = sum_m e^{t_j k_m}
  rbf:   W[j, n] = e^{-(q_n - t_j)^2/(2 s^2)}
         att[n] = (W.T gnum) / (W.T gden)
Key implementation trick: conv and partition-broadcast fuse into single
matmuls with host-replicated weight columns (k_bcast[j,m] = sum_c
wkrep[c,j] img[c,m]), in float32r (1 cycle/row, ~1e-4 rounding).
End-to-end rel err vs the exact softmax reference ~5e-3.
"""

import numpy as np

import concourse.bass as bass
import concourse.bacc as bacc
import concourse.mybir as mybir
from concourse.tile import TileContext
from concourse.bass_utils import run_bass_kernel_spmd

F32 = mybir.dt.float32
F32R = mybir.dt.float32r
BF16 = mybir.dt.bfloat16
AF = mybir.ActivationFunctionType
ALU = mybir.AluOpType

N = 4096          # pixels per image
NSL = 2048        # query pixels per core
NMT = 32          # key tiles of 128 (L-conv only)
NQT = 16          # query tiles of 128
P = 128
T = 128           # quadrature grid size
SIG_RATIO = 1.0   # sigma / grid spacing
LUMW = (0.299, 0.587, 0.114)
NCH = 4           # key chunks of 1024 in the grid build
CH = N // NCH


def build_nc(debug=False):
    nc = bacc.Bacc("TRN2", target_bir_lowering=False, debug=False,
                   num_devices=8)

    img = nc.declare_dram_parameter("img", [3, N], F32, isOutput=False)
    qimg = nc.declare_dram_parameter("qimg", [3, NSL], F32, isOutput=False)
    qimgT = nc.declare_dram_parameter("qimgT", [P, 3 * NQT], F32, isOutput=False)
    wkrep = nc.declare_dram_parameter("wkrep", [3, 3 * P], F32, isOutput=False)
    wvrep = nc.declare_dram_parameter("wvrep", [3, 3 * P], F32, isOutput=False)
    wqT = nc.declare_dram_parameter("wqT", [3, 3], F32, isOutput=False)
    lumrep = nc.declare_dram_parameter("lumrep", [3, 3], F32, isOutput=False)
    lumcol = nc.declare_dram_parameter("lumcol", [3, 1], F32, isOutput=False)
    wocol = nc.declare_dram_parameter("wocol", [P, 9], F32, isOutput=False)
    tcol = nc.declare_dram_parameter("tcol", [P, 1], F32, isOutput=False)
    tsig = nc.declare_dram_parameter("tsig", [P, 1], F32, isOutput=False)
    isq = nc.declare_dram_parameter("isq", [P, 1], F32, isOutput=False)
    out = nc.declare_dram_parameter("out", [P, 3 * NQT], F32, isOutput=True)
    if debug:
        dbg_g = nc.declare_dram_parameter("dbg_g", [P, 6], F32, isOutput=True)
        dbg_att = nc.declare_dram_parameter("dbg_att", [P, 3 * NQT], F32,
                                            isOutput=True)
        dbg_qp = nc.declare_dram_parameter("dbg_qp", [3, NSL], F32,
                                           isOutput=True)

    with TileContext(nc) as tc:
        with (
            tc.tile_pool(name="singles", bufs=1) as singles,
            tc.tile_pool(name="sbuf", bufs=2) as sb,
            tc.tile_pool(name="stile", bufs=3) as stile,
            tc.tile_pool(name="wtile", bufs=2) as wtile,
            tc.tile_pool(name="psum_row", bufs=1, space="PSUM") as psrow,
            tc.tile_pool(name="psum_bc", bufs=2, space="PSUM") as psbc,
            tc.tile_pool(name="psum_sm", bufs=2, space="PSUM") as pssm,
        ):
            # ---- load inputs ----
            img_sb = singles.tile([3, N], F32)
            nc.sync.dma_start(out=img_sb[:], in_=img[:])
            qimg_sb = singles.tile([3, NSL], F32)
            nc.sync.dma_start(out=qimg_sb[:], in_=qimg[:])
            qimgT_sb = singles.tile([P, 3 * NQT], F32)
            nc.sync.dma_start(out=qimgT_sb[:], in_=qimgT[:])
            wkrep_sb = singles.tile([3, 3 * P], F32)
            nc.sync.dma_start(out=wkrep_sb[:], in_=wkrep[:])
            wvrep_sb = singles.tile([3, 3 * P], F32)
            nc.sync.dma_start(out=wvrep_sb[:], in_=wvrep[:])
            wqT_sb = singles.tile([3, 3], F32)
            nc.sync.dma_start(out=wqT_sb[:], in_=wqT[:])
            lumrep_sb = singles.tile([3, 3], F32)
            nc.sync.dma_start(out=lumrep_sb[:], in_=lumrep[:])
            lumcol_sb = singles.tile([3, 1], F32)
            nc.sync.dma_start(out=lumcol_sb[:], in_=lumcol[:])
            wocol_sb = singles.tile([P, 9], F32)
            nc.sync.dma_start(out=wocol_sb[:], in_=wocol[:])
            tcol_sb = singles.tile([P, 1], F32)
            nc.sync.dma_start(out=tcol_sb[:], in_=tcol[:])
            tsig_sb = singles.tile([P, 1], F32)
            nc.sync.dma_start(out=tsig_sb[:], in_=tsig[:])
            isq_sb = singles.tile([P, 1], F32)
            nc.sync.dma_start(out=isq_sb[:], in_=isq[:])

            ones_sq = singles.tile([P, P], F32)
            nc.vector.memset(ones_sq[:], 1.0)

            warm_bf = singles.tile([P, 512], BF16)
            nc.vector.memset(warm_bf[:], 1.0)

            # float32r-rounded operands for the fused conv+broadcast matmuls
            img_r = singles.tile([3, N], F32R)
            nc.vector.tensor_copy(img_r[:], img_sb[:])
            qimg_r = singles.tile([3, NSL], F32R)
            nc.vector.tensor_copy(qimg_r[:], qimg_sb[:])
            wkrep_r = singles.tile([3, 3 * P], F32R)
            nc.vector.tensor_copy(wkrep_r[:], wkrep_sb[:])
            wvrep_r = singles.tile([3, 3 * P], F32R)
            nc.vector.tensor_copy(wvrep_r[:], wvrep_sb[:])
            wqT_r = singles.tile([3, 3], F32R)
            nc.vector.tensor_copy(wqT_r[:], wqT_sb[:])
            lumrep_r = singles.tile([3, 3], F32R)
            nc.vector.tensor_copy(lumrep_r[:], lumrep_sb[:])
            ones_r = singles.tile([1, P], F32R)
            nc.vector.tensor_copy(ones_r[:], ones_sq[0:1, :])

            # ---- luma stats over the full image (columns layout) ----
            psum_L = pssm.tile([P, NMT], F32, tag="sm")
            for mt in range(NMT):
                nc.tensor.matmul(psum_L[:, mt:mt + 1],
                                 lhsT=img_sb[:, mt * P:(mt + 1) * P],
                                 rhs=lumcol_sb[:], start=True, stop=True)
            L_sb = sb.tile([P, NMT], F32)
            nc.vector.tensor_copy(L_sb[:], psum_L[:])

            Lr = sb.tile([P, 1], F32)
            nc.vector.tensor_reduce(Lr[:], L_sb[:], axis=mybir.AxisListType.X,
                                    op=ALU.add)
            mu_ps = pssm.tile([P, 1], F32, tag="sm")
            nc.tensor.matmul(mu_ps[:], lhsT=ones_sq[:], rhs=Lr[:],
                             start=True, stop=True)
            mu_sb = sb.tile([P, 1], F32)
            nc.vector.tensor_scalar_mul(mu_sb[:], mu_ps[:], 1.0 / N)

            dltmp = sb.tile([P, NMT], F32)
            nc.vector.tensor_scalar(dltmp[:], L_sb[:], mu_sb[:, 0:1], None,
                                    op0=ALU.subtract)
            sr = sb.tile([P, 2], F32)
            nc.vector.tensor_reduce(sr[:, 0:1], dltmp[:],
                                    axis=mybir.AxisListType.X,
                                    op=ALU.add, apply_absolute_value=True)
            dl2 = sb.tile([P, NMT], F32)
            nc.vector.tensor_tensor(dl2[:], dltmp[:], dltmp[:], op=ALU.mult)
            nc.vector.tensor_reduce(sr[:, 1:2], dl2[:],
                                    axis=mybir.AxisListType.X, op=ALU.add)
            stats_ps = pssm.tile([P, 2], F32, tag="sm")
            nc.tensor.matmul(stats_ps[:], lhsT=ones_sq[:], rhs=sr[:],
                             start=True, stop=True)
            stats_sb = sb.tile([P, 2], F32)
            nc.vector.tensor_copy(stats_sb[:], stats_ps[:])
            s1sq = sb.tile([P, 1], F32)
            nc.vector.tensor_tensor(s1sq[:], stats_sb[:, 0:1],
                                    stats_sb[:, 0:1], op=ALU.mult)
            var_sb = sb.tile([P, 1], F32)
            nc.vector.scalar_tensor_tensor(var_sb[:], in0=s1sq[:],
                                           scalar=-1.0 / N,
                                           in1=stats_sb[:, 1:2],
                                           op0=ALU.mult, op1=ALU.add)
            nc.vector.tensor_scalar_mul(var_sb[:], var_sb[:], 1.0 / (N - 1))
            lnv = sb.tile([P, 1], F32)
            nc.scalar.activation(lnv[:], var_sb[:], AF.Ln)
            stdv = sb.tile([P, 1], F32)
            nc.scalar.activation(stdv[:], lnv[:], AF.Exp, scale=0.5)
            nc.vector.tensor_scalar_add(stdv[:], stdv[:], 1e-6)
            rinv = sb.tile([P, 1], F32)
            nc.vector.reciprocal(rinv[:], stdv[:])
            rinv_half = sb.tile([P, 1], F32)
            nc.vector.tensor_scalar_mul(rinv_half[:], rinv[:], 0.5)

            # PE warm-up right before the dense matmul phase (HAM gate)
            warm_ps = pssm.tile([P, 512], F32, tag="sm")
            for _ in range(11):
                nc.tensor.matmul(warm_ps[:], lhsT=warm_bf[:, 0:P],
                                 rhs=warm_bf[:], start=True, stop=True)

            # ---- per-head grid build: chunk-outer, head-inner for PE density
            att_sb = singles.tile([P, 3, NQT], F32)
            dparts = sb.tile([P, 3, NCH], F32)
            nparts = sb.tile([P, 3, NCH], F32)
            for ch in range(NCH):
                for h in range(3):
                    # fused conv+broadcast: kb[j, m] = k[h, m] for all j
                    kb_ps = psbc.tile([P, CH], F32, tag="bc")
                    for c4 in range(2):
                        off = ch * CH + c4 * 512
                        nc.tensor.matmul(
                            kb_ps[:, c4 * 512:(c4 + 1) * 512],
                            lhsT=wkrep_r[:, h * P:(h + 1) * P],
                            rhs=img_r[:, off:off + 512],
                            start=True, stop=True)
                    s_t = stile.tile([P, CH], BF16, tag="s")
                    nc.scalar.activation(s_t[:], kb_ps[:], AF.Exp,
                                         scale=tcol_sb[:, 0:1],
                                         accum_out=dparts[:, h, ch:ch + 1])
                    vb_ps = psbc.tile([P, CH], F32, tag="bc")
                    for c4 in range(2):
                        off = ch * CH + c4 * 512
                        nc.tensor.matmul(
                            vb_ps[:, c4 * 512:(c4 + 1) * 512],
                            lhsT=wvrep_r[:, h * P:(h + 1) * P],
                            rhs=img_r[:, off:off + 512],
                            start=True, stop=True)
                    junk = stile.tile([P, CH], BF16, tag="junk")
                    nc.vector.scalar_tensor_tensor(
                        junk[:], in0=s_t[:], scalar=1.0, in1=vb_ps[:],
                        op0=ALU.bypass, op1=ALU.mult,
                        accum_out=nparts[:, h, ch:ch + 1])

            # ---- gate + q' in row layout (two 1024-wide halves) ----
            qp_row = singles.tile([3, NSL], F32)
            for half in range(2):
                hs = half * 1024
                Lq_ps = psrow.tile([3, 1024], F32, tag="row")
                for c4 in range(2):
                    nc.tensor.matmul(
                        Lq_ps[:, c4 * 512:(c4 + 1) * 512],
                        lhsT=lumrep_r[:],
                        rhs=qimg_r[:, hs + c4 * 512:hs + (c4 + 1) * 512],
                        start=True, stop=True)
                dlq = sb.tile([3, 1024], F32, tag="dlq")
                nc.vector.tensor_scalar(dlq[:], Lq_ps[:], mu_sb[0:3, 0:1],
                                        None, op0=ALU.subtract)
                nc.vector.scalar_tensor_tensor(dlq[:], in0=dlq[:],
                                               scalar=-1.0, in1=dlq[:],
                                               op0=ALU.mult, op1=ALU.max)
                gate = sb.tile([3, 1024], F32, tag="gate")
                nc.scalar.activation(gate[:], dlq[:], AF.Tanh,
                                     scale=rinv_half[0:3, 0:1])
                nc.vector.tensor_scalar_mul(gate[:], gate[:], 0.5)
                nc.vector.tensor_scalar_add(gate[:], gate[:], 1.5)
                q_ps = psrow.tile([3, 1024], F32, tag="row")
                for c4 in range(2):
                    nc.tensor.matmul(
                        q_ps[:, c4 * 512:(c4 + 1) * 512],
                        lhsT=wqT_r[:],
                        rhs=qimg_r[:, hs + c4 * 512:hs + (c4 + 1) * 512],
                        start=True, stop=True)
                nc.vector.tensor_tensor(qp_row[:, hs:hs + 1024], q_ps[:],
                                        gate[:], op=ALU.mult)
            if debug:
                nc.sync.dma_start(out=dbg_qp[:], in_=qp_row[:])
            qp_row_r = singles.tile([3, NSL], F32R)
            nc.vector.tensor_copy(qp_row_r[:], qp_row[:])

            for h in range(3):
                g2f = sb.tile([P, 2], F32)
                nc.vector.tensor_reduce(g2f[:, 0:1], nparts[:, h, :],
                                        axis=mybir.AxisListType.X, op=ALU.add)
                nc.vector.tensor_reduce(g2f[:, 1:2], dparts[:, h, :],
                                        axis=mybir.AxisListType.X, op=ALU.add)
                g2_sb = sb.tile([P, 2], BF16)
                nc.vector.tensor_copy(g2_sb[:], g2f[:])
                if debug:
                    nc.sync.dma_start(out=dbg_g[:, 2 * h:2 * h + 2],
                                      in_=g2f[:])

                # q' row for this head to partition 0, then broadcast + W
                qrow_h = sb.tile([1, NSL], F32R, tag="qrh")
                nc.sync.dma_start(out=qrow_h[:], in_=qp_row_r[h:h + 1, :])
                z2 = wtile.tile([P, NSL], F32, tag="z2")
                for ch in range(2):
                    qb_ps = psbc.tile([P, 1024], F32, tag="bc")
                    for c4 in range(2):
                        off = ch * 1024 + c4 * 512
                        nc.tensor.matmul(
                            qb_ps[:, c4 * 512:(c4 + 1) * 512],
                            lhsT=ones_r[:],
                            rhs=qrow_h[0:1, off:off + 512],
                            start=True, stop=True)
                    nc.scalar.activation(z2[:, ch * 1024:(ch + 1) * 1024],
                                         qb_ps[:], AF.Square,
                                         scale=isq_sb[:, 0:1],
                                         bias=tsig_sb[:, 0:1])
                w_sb = wtile.tile([P, NSL], BF16, tag="w")
                nc.scalar.activation(w_sb[:], z2[:], AF.Exp, scale=-1.0)

                psum_att = pssm.tile([P, 2 * NQT], F32, tag="sm")
                for qt in range(NQT):
                    nc.tensor.matmul(psum_att[:, 2 * qt:2 * qt + 2],
                                     lhsT=w_sb[:, qt * P:(qt + 1) * P],
                                     rhs=g2_sb[:],
                                     start=True, stop=True)
                rden = sb.tile([P, NQT], F32)
                nc.vector.reciprocal(
                    rden[:],
                    psum_att[:].rearrange("p (q two) -> p q two", two=2)[:, :, 1])
                nc.vector.tensor_tensor(
                    att_sb[:, h, :],
                    psum_att[:].rearrange("p (q two) -> p q two", two=2)[:, :, 0],
                    rden[:], op=ALU.mult)

            if debug:
                nc.sync.dma_start(out=dbg_att[:],
                                  in_=att_sb[:].rearrange("p h q -> p (h q)"))

            # ---- output mix (wo), residual, clip -- column layout ----
            out_sb = singles.tile([P, 3, NQT], F32)
            for cch in range(3):
                mix = sb.tile([P, NQT], F32, tag="mix")
                nc.vector.tensor_scalar(mix[:], att_sb[:, 0, :],
                                        wocol_sb[:, 3 * cch:3 * cch + 1], None,
                                        op0=ALU.mult)
                for h in (1, 2):
                    nc.vector.scalar_tensor_tensor(
                        mix[:], in0=att_sb[:, h, :],
                        scalar=wocol_sb[:, 3 * cch + h:3 * cch + h + 1],
                        in1=mix[:], op0=ALU.mult, op1=ALU.add)
                nc.vector.tensor_tensor(
                    mix[:], mix[:],
                    qimgT_sb[:, cch * NQT:(cch + 1) * NQT], op=ALU.add)
                nc.vector.tensor_scalar_max(mix[:], mix[:], 0.0)
                nc.vector.tensor_scalar_min(out_sb[:, cch, :], mix[:], 1.0)
            nc.sync.dma_start(out=out[:],
                              in_=out_sb[:].rearrange("p c q -> p (c q)"))

    nc.finalize()
    return nc


_NC_CACHE = {}


def _get_nc(debug=False):
    key = ("dbg" if debug else "nc")
    if key not in _NC_CACHE:
        _NC_CACHE[key] = build_nc(debug)
    return _NC_CACHE[key]


def make_in_maps(rgb, wq, wk, wv, wo):
    x = np.ascontiguousarray(rgb.reshape(4, 3, N)).astype(np.float32)
    lumw = np.array(LUMW, dtype=np.float32)
    wkrep = np.concatenate([np.tile(wk[h][:, None], (1, P)) for h in range(3)],
                           axis=1).astype(np.float32)
    wvrep = np.concatenate([np.tile(wv[h][:, None], (1, P)) for h in range(3)],
                           axis=1).astype(np.float32)
    wqT = np.ascontiguousarray(wq.T).astype(np.float32)
    lumrep = np.tile(lumw[:, None], (1, 3)).astype(np.float32)
    lumcol = lumw.reshape(3, 1).astype(np.float32)
    wocol = np.tile(wo.reshape(1, 9), (P, 1)).astype(np.float32)
    # runtime quadrature grid: |q'| <= 2 * max_h sum_c |wq[h,c]| (+ margin)
    R = 2.0 * float(np.abs(wq).sum(axis=1).max()) + 1.0
    hg = 2.0 * R / (T - 1)
    sig = SIG_RATIO * hg
    isqv = 1.0 / (sig * np.sqrt(2.0))
    tg = (-R + np.arange(T) * hg).astype(np.float32)
    tcolv = tg.reshape(P, 1).astype(np.float32)
    tsig = (-tg * isqv).reshape(P, 1).astype(np.float32)
    isqa = np.full((P, 1), isqv, np.float32)

    in_maps = []
    for j in range(8):
        b, half = j // 2, j % 2
        sl = slice(half * NSL, (half + 1) * NSL)
        qs = x[b][:, sl]
        qT = np.ascontiguousarray(
            qs.reshape(3, NQT, P).transpose(2, 0, 1).reshape(P, 3 * NQT))
        in_maps.append({
            "img": x[b],
            "qimg": np.ascontiguousarray(qs),
            "qimgT": qT.astype(np.float32),
            "wkrep": wkrep,
            "wvrep": wvrep,
            "wqT": wqT,
            "lumrep": lumrep,
            "lumcol": lumcol,
            "wocol": wocol,
            "tcol": tcolv,
            "tsig": tsig,
            "isq": isqa,
        })
    return in_maps


def run(rgb, wq, wk, wv, wo, trace=False, debug=False):
    nc = _get_nc(debug)
    in_maps = make_in_maps(rgb, wq, wk, wv, wo)
    res = run_bass_kernel_spmd(nc, in_maps, core_ids=list(range(8)),
                               trace=trace)
    y = np.zeros((4, 3, N), dtype=np.float32)
    for j in range(8):
        b, half = j // 2, j % 2
        sl = slice(half * NSL, (half + 1) * NSL)
        o = res.results[j]["out"]
        y[b][:, sl] = o.reshape(P, 3, NQT).transpose(1, 2, 0).reshape(3, NSL)
    return y.reshape(4, 3, 64, 64), res


def kernel(**inputs):
    y, _ = run(inputs["rgb"], inputs["wq"], inputs["wk"], inputs["wv"],
               inputs["wo"])
    return y
